# revision 23
# baseline (speedup 1.0000x reference)
"""Bidirectional 2-layer LSTM (B=256, T=128, EMB=256, HS=512, VS=64) on 8 trn2 cores.

Sharding: core = (batch-half bh, direction d, gate-half g).
Each core runs the full 2-layer recurrence for its 128 batch rows and
direction, computing HALF the gate/hidden dims (hid [g*256,(g+1)*256) of
both layers); gate-half pairs AllGather their h-halves every step.

Per-iteration u (uniform, u = 0..T):
  - 26 matmuls (K<=128, M=128, N=512) accumulate L0 gates (step u) and
    L1 gates (step u-1) into one PSUM tile [128, 2048] =
    [i0 f0 o0 g0 | i1 f1 o1 g1] (own hid-half, 256 each).
  - 6 fused elementwise ops compute BOTH layers' LSTM cells via strided
    3-D APs (sigmoid, tanh(g), i*g|f*c, add, tanh(c), o*tc).
  - H [128, 512] = [H0own|H1own] -> dram -> pair AllGather -> load full
    H [128,1024] -> ONE dma_start_transpose -> hT blocks for next step.
Tail: dir-pairs AllReduce(add) the late h-slot window; partner stream =
sum - own (keeps SPMD code uniform under time reversal); each core
computes compress+fc for its own-scan steps [0, T/2) (fwd cores cover
original t in [0,T/2), bwd cores cover [T/2, T)).
"""

import os
import sys
from contextlib import ExitStack

import numpy as np
import ml_dtypes

for _p in ("/opt/trn_rl_repo",):
    if _p not in sys.path and os.path.isdir(_p):
        sys.path.insert(0, _p)

os.environ.setdefault("JAX_COMPILATION_CACHE_DIR", "/tmp/jaxcache")
os.environ.setdefault("JAX_PERSISTENT_CACHE_MIN_COMPILE_TIME_SECS", "1")

B, T, VS, EMB, HS = 256, 128, 64, 256, 512
NCORES = 8
BC = 128           # batch rows per core
HH = 256           # hid per gate-half

BF16 = ml_dtypes.bfloat16

# core = bh*4 + d*2 + g
GATE_PAIRS = [[0, 1], [2, 3], [4, 5], [6, 7]]   # (bh,d,g0) <-> (bh,d,g1)
DIR_PAIRS = [[0, 2], [1, 3], [4, 6], [5, 7]]    # (bh,f,g) <-> (bh,b,g)

# dmat block j of transposed full-H [128, 8, 128]:
# HF = [H0(g0) H1(g0) H0(g1) H1(g1)] in 256-col slabs ->
# blocks: [h0c0 h0c1 h1c0 h1c1 h0c2 h0c3 h1c2 h1c3]
BH0 = [0, 1, 4, 5]
BH1 = [2, 3, 6, 7]


def build_program(t_steps=T, repeat=1, with_b1=False, fp8=False, tail8=False):
    import concourse.bass as bass  # noqa: F401
    import concourse.mybir as mybir
    import concourse.tile as tile
    from concourse import bacc

    f32 = mybir.dt.float32
    bf16 = mybir.dt.bfloat16
    fp8e4 = mybir.dt.float8e4
    DR = mybir.MatmulPerfMode.DoubleRow
    AF = mybir.ActivationFunctionType
    Tn = t_steps
    TS = Tn // 2                  # own-scan tail steps per core
    NG = TS // 4                  # tail groups (4 steps = 512 rows each)

    nc = bacc.Bacc()

    # ---- I/O ----
    ohT = nc.dram_tensor("ohT", [64, (Tn + 1) * BC], bf16, kind="ExternalInput")
    g0tab = nc.dram_tensor("g0tab", [64, 1024], bf16, kind="ExternalInput")
    if fp8:
        # DoubleRow pair tiles: [A n0 | B n0 | A n1 | B n1] per chunk pair
        wh0 = nc.dram_tensor("wh0", [2, 128, 2048], fp8e4, kind="ExternalInput")
        wx1 = nc.dram_tensor("wx1", [2, 128, 2048], fp8e4, kind="ExternalInput")
        wh1 = nc.dram_tensor("wh1", [2, 128, 2048], fp8e4, kind="ExternalInput")
    else:
        wh0 = nc.dram_tensor("wh0", [4, 128, 1024], bf16, kind="ExternalInput")
        wx1 = nc.dram_tensor("wx1", [4, 128, 1024], bf16, kind="ExternalInput")
        wh1 = nc.dram_tensor("wh1", [4, 128, 1024], bf16, kind="ExternalInput")
    wco = nc.dram_tensor("wco", [4, 128, 256], bf16, kind="ExternalInput")
    wcp = nc.dram_tensor("wcp", [4, 128, 256], bf16, kind="ExternalInput")
    fct = nc.dram_tensor("fct", [128, 128], bf16, kind="ExternalInput")
    cbias = nc.dram_tensor("cbias", [128, 2], f32, kind="ExternalInput")
    fbias = nc.dram_tensor("fbias", [64, 1], f32, kind="ExternalInput")
    if with_b1:
        b1row = nc.dram_tensor("b1row", [1, 1024], bf16, kind="ExternalInput")
    logT = nc.dram_tensor("logT", [64, TS * BC], bf16, kind="ExternalOutput")

    # internal dram
    logP = nc.dram_tensor("logP", [64, TS * BC], bf16)
    logS = nc.dram_tensor("logS", [64, TS * BC], bf16)
    hin = nc.dram_tensor("hin", [Tn + 1, 128, 512], bf16)
    hout = nc.dram_tensor("hout", [Tn + 1, 2, 128, 512], bf16)
    arout = nc.dram_tensor("arout", [TS, 2, 128, 512], bf16)

    if os.environ.get("BLSTM_NULL", "0") == "1":
        with tile.TileContext(nc) as tc, ExitStack() as ctx:
            pool = ctx.enter_context(tc.tile_pool(name="np", bufs=1))
            z = pool.tile([64, 512], bf16, name="z")
            nc.vector.memset(z, 0.0)
            nc.sync.dma_start(out=logT[:, 0:512], in_=z)
        nc.finalize()
        return nc

    with tile.TileContext(nc) as tc, ExitStack() as ctx:
        wpool = ctx.enter_context(tc.tile_pool(name="weights", bufs=1))
        spool = ctx.enter_context(tc.tile_pool(name="state", bufs=1))
        work = ctx.enter_context(tc.tile_pool(name="work", bufs=1))
        gpool = ctx.enter_context(tc.tile_pool(name="gp", bufs=1, space="PSUM"))
        auxp = ctx.enter_context(tc.tile_pool(name="auxp", bufs=1, space="PSUM"))

        # ---- load weights ----
        def load(dram, n, cols, tag):
            tiles = []
            for k in range(n):
                t_ = wpool.tile([128, cols], bf16, tag=f"{tag}{k}", name=f"{tag}{k}")
                nc.sync.dma_start(out=t_, in_=dram[k])
                tiles.append(t_)
            return tiles

        ohT_s = wpool.tile([64, (Tn + 1) * BC], bf16, tag="ohT")
        nc.sync.dma_start(out=ohT_s, in_=ohT[:, :])
        g0tab_s = wpool.tile([64, 1024], bf16, tag="g0tab")
        nc.sync.dma_start(out=g0tab_s, in_=g0tab[:, :])

        def load8(dram, n, cols, tag):
            tiles = []
            for k in range(n):
                t_ = wpool.tile([128, cols], fp8e4, tag=f"{tag}{k}", name=f"{tag}{k}")
                nc.sync.dma_start(out=t_, in_=dram[k])
                tiles.append(t_)
            return tiles

        if fp8:
            wh0_s = load8(wh0, 2, 2048, "wh0")
            wx1_s = load8(wx1, 2, 2048, "wx1")
            wh1_s = load8(wh1, 2, 2048, "wh1")
        else:
            wh0_s = load(wh0, 4, 1024, "wh0")
            wx1_s = load(wx1, 4, 1024, "wx1")
            wh1_s = load(wh1, 4, 1024, "wh1")
        wco_s = load(wco, 4, 256, "wco")
        wcp_s = load(wcp, 4, 256, "wcp")
        fct_s = wpool.tile([128, 128], bf16, tag="fct")
        nc.sync.dma_start(out=fct_s, in_=fct[:, :])
        cbias_s = wpool.tile([128, 2], f32, tag="cbias")
        nc.sync.dma_start(out=cbias_s, in_=cbias[:, :])
        fbias_s = wpool.tile([64, 1], f32, tag="fbias")
        nc.sync.dma_start(out=fbias_s, in_=fbias[:, :])
        if with_b1:
            b1_s = wpool.tile([1, 1024], bf16, tag="b1row")
            nc.sync.dma_start(out=b1_s, in_=b1row[:, :])
            ones_s = wpool.tile([1, 128], bf16, tag="ones")
            nc.vector.memset(ones_s, 1.0)

        # ---- state ----
        # X: [G0(256) c0(256) G1(256) c1(256)] fp32
        X = spool.tile([128, 1024], f32, tag="X")
        hT_ring = [spool.tile([128, 1024], bf16, tag=f"hT{i}", name=f"hT{i}")
                   for i in range(3)]
        if fp8:
            hT8_ring = [spool.tile([128, 1024], fp8e4, tag=f"hT8{i}",
                                   name=f"hT8{i}") for i in range(3)]

        def init_state():
            nc.vector.memset(X, 0.0)
            nc.vector.memset(hT_ring[2], 0.0)
            if fp8:
                nc.vector.memset(hT8_ring[2], 0.0)

        def gates_mms(gp, u, hT, mode="full"):
            """Gate matmuls: L0 gates (step u) into cols 0:1024,
            L1 gates (step u-1) into cols 1024:2048.
            mode="first": h(-1)=0 -> only x-part mms + memset L1 region.
            mode="last": L0 output unused -> memset L0 region, L1 mms only."""
            xstat = (ohT_s[:, u * BC:(u + 1) * BC], g0tab_s, None)
            if fp8:
                hv = hT.rearrange("p (j b) -> p j b", j=8)
                # DoubleRow pairs: h0 = blocks (0,1),(4,5); h1 = (2,3),(6,7)
                reg0 = [xstat]
                reg0 += [(hv[:, 4 * P:4 * P + 2, :], wh0_s[P], DR)
                         for P in (0, 1)]
                reg1 = [(hv[:, 4 * P:4 * P + 2, :], wx1_s[P], DR)
                        for P in (0, 1)]
                reg1 += [(hv[:, 4 * P + 2:4 * P + 4, :], wh1_s[P], DR)
                         for P in (0, 1)]
            else:
                reg0 = [xstat]
                reg0 += [(hT[:, BH0[kc] * 128:BH0[kc] * 128 + 128],
                          wh0_s[kc], None) for kc in range(4)]
                reg1 = [(hT[:, BH0[kc] * 128:BH0[kc] * 128 + 128],
                         wx1_s[kc], None) for kc in range(4)]
                reg1 += [(hT[:, BH1[kc] * 128:BH1[kc] * 128 + 128],
                          wh1_s[kc], None) for kc in range(4)]
            if with_b1:
                reg1.append((ones_s, b1_s, None))
            if mode == "first" and not with_b1:
                # x-part only; L1 gates (step -1) are all zero
                reg0 = [xstat]
                nc.vector.memset(gp[:, 1024:2048], 0.0)
                reg1 = []
            elif mode == "last":
                nc.vector.memset(gp[:, 0:1024], 0.0)
                reg0 = []
            for base, stats in ((0, reg0), (1024, reg1)):
                nk = len(stats)
                for kid, (lhs, w, pm) in enumerate(stats):
                    for n in (0, 1):
                        if pm is DR:
                            rhs = w.rearrange("p (t i n) -> p t i n",
                                              t=2, i=2)[:, n]
                        else:
                            rhs = w[:, 512 * n: 512 * n + 512]
                        nc.tensor.matmul(
                            gp[:, base + 512 * n: base + 512 * n + 512],
                            lhsT=lhs,
                            rhs=rhs,
                            start=(kid == 0),
                            stop=(kid == nk - 1),
                            perf_mode=pm,
                            tile_position=(0, 0),
                        )

        def cell(gp):
            """Fused 2-layer cell; returns H [128, 512] = [H0own|H1own]."""
            gv = gp.rearrange("p (j c) -> p j c", j=2)       # [128, 2, 1024]
            S = work.tile([128, 1536], bf16, tag="S")
            Sv = S.rearrange("p (j c) -> p j c", j=2)        # [128, 2, 768]
            nc.scalar.activation(Sv, gv[:, :, 0:768], AF.Sigmoid)
            Xv = X.rearrange("p (j c) -> p j c", j=2)        # [128, 2, 512]
            nc.scalar.activation(Xv[:, :, 0:256], gv[:, :, 768:1024], AF.Tanh)
            P = work.tile([128, 1024], f32, tag="P")
            Pv = P.rearrange("p (j c) -> p j c", j=2)        # [128, 2, 512]
            nc.vector.tensor_mul(Pv, Sv[:, :, 0:512], Xv)
            nc.vector.tensor_add(Xv[:, :, 256:512], Pv[:, :, 0:256],
                                 Pv[:, :, 256:512])
            TC = work.tile([128, 512], bf16, tag="TC")
            TCv = TC.rearrange("p (j c) -> p j c", j=2)      # [128, 2, 256]
            nc.scalar.activation(TCv, Xv[:, :, 256:512], AF.Tanh)
            H = work.tile([128, 512], bf16, tag="H")
            Hv = H.rearrange("p (j c) -> p j c", j=2)
            nc.vector.tensor_mul(Hv, Sv[:, :, 512:768], TCv)
            return H

        def emit_recurrence():
            init_state()
            for u in range(Tn + 1):
                gp = gpool.tile([128, 2048], f32, tag="gp", name="gp")
                ring = hT8_ring if fp8 else hT_ring
                mode = "first" if u == 0 else ("last" if u == Tn else "full")
                gates_mms(gp, u, ring[(u - 1) % 3] if u > 0 else ring[2], mode)
                H = cell(gp)
                nc.sync.dma_start(out=hin[u], in_=H)
                nc.gpsimd.collective_compute(
                    "AllGather", mybir.AluOpType.bypass,
                    replica_groups=GATE_PAIRS,
                    ins=[hin[u]], outs=[hout[u]])
                HF = work.tile([128, 1024], bf16, tag="HF")
                nc.sync.dma_start(out=HF, in_=hout[u].rearrange("j p c -> p j c"))
                nc.sync.dma_start_transpose(
                    out=hT_ring[u % 3].rearrange("p (j b) -> p j b", j=8),
                    in_=HF)
                if fp8:
                    nc.vector.tensor_copy(hT8_ring[u % 3], hT_ring[u % 3])

        def emit_tail():
            # AllReduce the late window: slots [TS+1, Tn+1) (TS slots).
            nc.gpsimd.collective_compute(
                "AllReduce", mybir.AluOpType.add,
                replica_groups=DIR_PAIRS,
                ins=[hin_window()], outs=[arout[:, :, :, :]])
            for i in range(NG):
                emit_tail_group(i)
            nc.gpsimd.collective_compute(
                "AllReduce", mybir.AluOpType.add,
                replica_groups=GATE_PAIRS,
                ins=[logP[:, :]], outs=[logS[:, :]])
            nc.sync.dma_start(out=logT[:, :], in_=logS[:, :])

        def hin_window():
            # copy hout slots [TS+1 : Tn+1) to arin via dram->dram dma?
            # cheaper: AllReduce directly on a hout slice.
            return hout[TS + 1: Tn + 1]

        def emit_tail_group(i):
            # own-scan steps s = 4i .. 4i+3; 512 rows.
            # Load FULL h slots [2,128,512] -> [128, 1024] per step
            # (h1 blocks selected post-transpose via BH1 strides).
            HT1 = work.tile([128, 4096], bf16, tag="HT1")   # own-dir rows
            ART = work.tile([128, 4096], bf16, tag="ART")   # fwd+bwd sum
            OWT = work.tile([128, 4096], bf16, tag="OWT")   # own window
            for k in range(4):
                s = 4 * i + k
                cs = slice(k * 1024, (k + 1) * 1024)
                rr = lambda t_: t_.rearrange("j p c -> p j c")
                nc.sync.dma_start(out=HT1[:, cs], in_=rr(hout[s + 1]))
                nc.sync.dma_start(out=ART[:, cs], in_=rr(arout[TS - 1 - s]))
                nc.sync.dma_start(out=OWT[:, cs], in_=rr(hout[Tn - s]))
            PRT = work.tile([128, 4096], bf16, tag="PRT")   # partner rows
            nc.vector.tensor_sub(PRT, ART, OWT)
            hTo = work.tile([128, 4096], bf16, tag="hTo")
            nc.sync.dma_start_transpose(
                out=hTo.rearrange("p (j b) -> p j b", j=32), in_=HT1)
            hTp = work.tile([128, 4096], bf16, tag="hTp")
            nc.sync.dma_start_transpose(
                out=hTp.rearrange("p (j b) -> p j b", j=32), in_=PRT)
            # compress matmuls for this core's comp-HALF: 2 cc chunks
            PT = auxp.tile([128, 1024], f32, tag="aux", name="PT")
            hToV = hTo.rearrange("p (s j b) -> p s j b", s=4, j=8)
            hTpV = hTp.rearrange("p (s j b) -> p s j b", s=4, j=8)
            for cc in range(2):
                for ki, (wt, hv) in enumerate(
                        [(wco_s, hToV), (wcp_s, hTpV)]):
                    for kc in range(4):
                        nc.tensor.matmul(
                            PT[:, cc * 512:(cc + 1) * 512],
                            lhsT=wt[kc][:, cc * 128:(cc + 1) * 128],
                            rhs=hv[:, :, BH1[kc], :],
                            start=(ki == 0 and kc == 0),
                            stop=(ki == 1 and kc == 3),
                            tile_position=(0, 0),
                        )
            C = work.tile([128, 1024], bf16, tag="C")
            for cc in range(2):
                nc.scalar.activation(
                    C[:, cc * 512:(cc + 1) * 512],
                    PT[:, cc * 512:(cc + 1) * 512],
                    AF.Tanh, bias=cbias_s[:, cc:cc + 1])
            lg = auxp.tile([64, 512], f32, tag="aux", name="lg")
            for cc in range(2):
                nc.tensor.matmul(
                    lg,
                    lhsT=fct_s[:, cc * 64:(cc + 1) * 64],
                    rhs=C[:, cc * 512:(cc + 1) * 512],
                    start=(cc == 0), stop=(cc == 1),
                    tile_position=(0, 0))
            lgs = work.tile([64, 512], bf16, tag="lgs")
            nc.scalar.activation(lgs, lg, AF.Identity, bias=fbias_s[:, 0:1])
            nc.sync.dma_start(out=logP[:, 512 * i: 512 * (i + 1)], in_=lgs)

        for _ in range(repeat):
            emit_recurrence()
            emit_tail()

    nc.finalize()
    return nc


_prog_cache = {}


def _get_program(key):
    if key not in _prog_cache:
        _prog_cache[key] = build_program(*key)
    return _prog_cache[key]


def _drpackc(w4):
    # [4, 128, 512] -> pairs [2, 128, 1024]: per cc: [A_cc(128) | B_cc(128)]
    FP8 = ml_dtypes.float8_e4m3
    out = np.empty((2, 128, 1024), np.float32)
    for P in (0, 1):
        A, Bc = w4[2 * P], w4[2 * P + 1]
        cols = []
        for cc in range(4):
            cols += [A[:, cc * 128:(cc + 1) * 128], Bc[:, cc * 128:(cc + 1) * 128]]
        out[P] = np.concatenate(cols, axis=1)
    return out.astype(FP8)


def _gate_perm_half(g):
    """Rows of W (gate dim 2048, blocks [i,f,g,o] of 512) for half g in
    cell order [i f o g] x hid [g*256,(g+1)*256)."""
    perm = []
    for blk in (0, 1, 3, 2):   # i, f, o, g
        base = 512 * blk + HH * g
        perm.extend(range(base, base + HH))
    return np.array(perm)


def _prep_core_inputs(x, emb_table, inputs, bh, d, g, t_steps=T, fp8=False, tail8=False):
    perm = _gate_perm_half(g)
    Tn = t_steps
    xq = np.asarray(x[bh * BC:(bh + 1) * BC, :Tn]).astype(np.int64)
    if d == 1:
        xq = xq[:, ::-1]
    xs = xq.T.reshape(-1)                       # [Tn*BC] scan-order tokens
    ohv = np.zeros((64, (Tn + 1) * BC), dtype=np.float32)
    ohv[xs, np.arange(Tn * BC)] = 1.0           # last step stays zero

    W0 = np.asarray(inputs["W_f0"] if d == 0 else inputs["W_b0"])
    b0 = np.asarray(inputs["b_f0"] if d == 0 else inputs["b_b0"])
    W1 = np.asarray(inputs["W_f1"] if d == 0 else inputs["W_b1"])
    b1 = np.asarray(inputs["b_f1"] if d == 0 else inputs["b_b1"])
    W0h = W0[perm].astype(np.float32)           # [1024, 768]
    W1h = W1[perm].astype(np.float32)           # [1024, 1024]
    g0v = np.asarray(emb_table, np.float32) @ W0h[:, :EMB].T + b0[perm][None, :]
    wh0v = W0h[:, EMB:].T.reshape(4, 128, 1024)
    wx1v = W1h[:, :HS].T.reshape(4, 128, 1024)
    wh1v = W1h[:, HS:].T.reshape(4, 128, 1024)

    Wc = np.asarray(inputs["compress_W"], np.float32)     # [512, 1024]
    wc_own = Wc[:, d * HS:(d + 1) * HS]                   # own direction
    wc_prt = Wc[:, (1 - d) * HS:(2 - d) * HS]
    # comp-HALF split across gate-half cores: this core handles comp
    # dims [g*256, (g+1)*256)
    wcov = wc_own.T.reshape(4, 128, 512)[:, :, g * 256:(g + 1) * 256]
    wcpv = wc_prt.T.reshape(4, 128, 512)[:, :, g * 256:(g + 1) * 256]
    wcov = np.ascontiguousarray(wcov)
    wcpv = np.ascontiguousarray(wcpv)

    if fp8:
        FP8 = ml_dtypes.float8_e4m3

        def drpack(w4):
            # [4, 128, 1024] -> pairs [2, 128, 2048]: [A n0 | B n0 | A n1 | B n1]
            out = np.empty((2, 128, 2048), np.float32)
            for P in (0, 1):
                A, Bc = w4[2 * P], w4[2 * P + 1]
                out[P] = np.concatenate(
                    [A[:, :512], Bc[:, :512], A[:, 512:], Bc[:, 512:]], axis=1)
            return out.astype(FP8)

        wh0v, wx1v, wh1v = drpack(wh0v), drpack(wx1v), drpack(wh1v)
    if tail8:
        wcov, wcpv = _drpackc(wcov), _drpackc(wcpv)
    # fcT chunks for this core's comp-half: 2 chunks [128, 64] in cols
    fctv = np.ascontiguousarray(
        np.asarray(inputs["fc_W"], np.float32).T[g * 256:(g + 1) * 256]
        .reshape(2, 128, 64).transpose(1, 0, 2).reshape(128, 128))
    cbv = np.ascontiguousarray(
        np.asarray(inputs["compress_b"], np.float32)
        [g * 256:(g + 1) * 256].reshape(2, 128).T)
    # fc bias contributed once per gate-pair (g0 core only)
    fbv = (np.asarray(inputs["fc_b"], np.float32).reshape(64, 1)
           if g == 0 else np.zeros((64, 1), np.float32))

    wdt = (lambda a: a) if fp8 else (lambda a: a.astype(BF16))
    cdt = (lambda a: a) if tail8 else (lambda a: a.astype(BF16))
    inmap = {
        "ohT": ohv.astype(BF16),
        "g0tab": g0v.astype(BF16),
        "wh0": wdt(wh0v),
        "wx1": wdt(wx1v),
        "wh1": wdt(wh1v),
        "wco": cdt(wcov),
        "wcp": cdt(wcpv),
        "fct": fctv.astype(BF16),
        "cbias": np.ascontiguousarray(cbv),
        "fbias": fbv,
    }
    if np.any(b1):
        inmap["b1row"] = b1[perm].reshape(1, 1024).astype(BF16)
    return inmap


def _run(inputs, trace=False, t_steps=T):
    from concourse.bass_utils import run_bass_kernel_spmd

    x = np.asarray(inputs["x"])
    emb_table = np.asarray(inputs["emb_table"], dtype=np.float32)
    with_b1 = bool(np.any(np.asarray(inputs["b_f1"]))
                   or np.any(np.asarray(inputs["b_b1"])))
    rep = int(os.environ.get("BLSTM_REPEAT", "1"))
    fp8 = os.environ.get("BLSTM_FP8", "0") == "1"
    tail8 = os.environ.get("BLSTM_FP8_TAIL", "0") == "1"
    nc = _get_program((t_steps, rep, with_b1, fp8, tail8))

    in_maps = []
    for core in range(NCORES):
        bh, d, g = core // 4, (core % 4) // 2, core % 2
        im = _prep_core_inputs(x, emb_table, inputs, bh, d, g, t_steps, fp8, tail8)
        if with_b1 and "b1row" not in im:
            im["b1row"] = np.zeros((1, 1024), dtype=BF16)
        in_maps.append(im)

    res = run_bass_kernel_spmd(nc, in_maps, core_ids=list(range(NCORES)),
                               trace=trace)

    Tn = t_steps
    TS = Tn // 2
    out = np.empty((B, Tn, VS), dtype=np.float32)
    for bh in (0, 1):
        logF = res.results[bh * 4 + 0]["logT"].astype(np.float32)
        logB = res.results[bh * 4 + 2]["logT"].astype(np.float32)
        bsl = slice(bh * BC, (bh + 1) * BC)
        # fwd core: scan u = orig t in [0, TS)
        out[bsl, :TS] = logF.reshape(VS, TS, BC).transpose(2, 1, 0)
        # bwd core: scan u = orig Tn-1-u; its rows cover orig [TS, Tn)
        ob = logB.reshape(VS, TS, BC).transpose(2, 1, 0)   # [BC, u, VS]
        out[bsl, TS:] = ob[:, ::-1]
    return out, res


def kernel(**inputs):
    out, _ = _run(inputs, trace=False)
    return out


def kernel_profiled(**inputs):
    out, res = _run(inputs, trace=True)
    return out, res


# revision 24
# speedup vs baseline: 1.1974x; 1.1974x over previous
"""Bidirectional 2-layer LSTM (B=256, T=128, EMB=256, HS=512, VS=64) on 8 trn2 cores.

Sharding: core = (batch-half bh, direction d, gate-half g).
Each core runs the full 2-layer recurrence for its 128 batch rows and
direction, computing HALF the gate/hidden dims (hid [g*256,(g+1)*256) of
both layers); gate-half pairs AllGather their h-halves every step.

Per-iteration u (uniform, u = 0..T):
  - 26 matmuls (K<=128, M=128, N=512) accumulate L0 gates (step u) and
    L1 gates (step u-1) into one PSUM tile [128, 2048] =
    [i0 f0 o0 g0 | i1 f1 o1 g1] (own hid-half, 256 each).
  - 6 fused elementwise ops compute BOTH layers' LSTM cells via strided
    3-D APs (sigmoid, tanh(g), i*g|f*c, add, tanh(c), o*tc).
  - H [128, 512] = [H0own|H1own] -> dram -> pair AllGather -> load full
    H [128,1024] -> ONE dma_start_transpose -> hT blocks for next step.
Tail: dir-pairs AllReduce(add) the late h-slot window; partner stream =
sum - own (keeps SPMD code uniform under time reversal); each core
computes compress+fc for its own-scan steps [0, T/2) (fwd cores cover
original t in [0,T/2), bwd cores cover [T/2, T)), split along the comp
dimension across gate-half cores via per-core weight data (2 of 4 comp
chunks each); partial logits are summed by a final bf16 AllReduce over
gate pairs. Boundary iterations are trimmed (u=0: x-part matmuls only
+ PSUM memset; u=T: L0 region memset, output discarded).
"""

import os
import sys
from contextlib import ExitStack

import numpy as np
import ml_dtypes

for _p in ("/opt/trn_rl_repo",):
    if _p not in sys.path and os.path.isdir(_p):
        sys.path.insert(0, _p)

os.environ.setdefault("JAX_COMPILATION_CACHE_DIR", "/tmp/jaxcache")
os.environ.setdefault("JAX_PERSISTENT_CACHE_MIN_COMPILE_TIME_SECS", "1")

B, T, VS, EMB, HS = 256, 128, 64, 256, 512
NCORES = 8
BC = 128           # batch rows per core
HH = 256           # hid per gate-half

BF16 = ml_dtypes.bfloat16

# core = bh*4 + d*2 + g
GATE_PAIRS = [[0, 1], [2, 3], [4, 5], [6, 7]]   # (bh,d,g0) <-> (bh,d,g1)
DIR_PAIRS = [[0, 2], [1, 3], [4, 6], [5, 7]]    # (bh,f,g) <-> (bh,b,g)

# dmat block j of transposed full-H [128, 8, 128]:
# HF = [H0(g0) H1(g0) H0(g1) H1(g1)] in 256-col slabs ->
# blocks: [h0c0 h0c1 h1c0 h1c1 h0c2 h0c3 h1c2 h1c3]
BH0 = [0, 1, 4, 5]
BH1 = [2, 3, 6, 7]


def build_program(t_steps=T, repeat=1, with_b1=False, fp8=False, tail8=False):
    import concourse.bass as bass  # noqa: F401
    import concourse.mybir as mybir
    import concourse.tile as tile
    from concourse import bacc

    f32 = mybir.dt.float32
    bf16 = mybir.dt.bfloat16
    fp8e4 = mybir.dt.float8e4
    DR = mybir.MatmulPerfMode.DoubleRow
    AF = mybir.ActivationFunctionType
    Tn = t_steps
    TS = Tn // 2                  # own-scan tail steps per core
    NG = TS // 4                  # tail groups (4 steps = 512 rows each)

    nc = bacc.Bacc()

    # ---- I/O ----
    ohT = nc.dram_tensor("ohT", [64, (Tn + 1) * BC], bf16, kind="ExternalInput")
    g0tab = nc.dram_tensor("g0tab", [64, 1024], bf16, kind="ExternalInput")
    if fp8:
        # DoubleRow pair tiles: [A n0 | B n0 | A n1 | B n1] per chunk pair
        wh0 = nc.dram_tensor("wh0", [2, 128, 2048], fp8e4, kind="ExternalInput")
        wx1 = nc.dram_tensor("wx1", [2, 128, 2048], fp8e4, kind="ExternalInput")
        wh1 = nc.dram_tensor("wh1", [2, 128, 2048], fp8e4, kind="ExternalInput")
    else:
        wh0 = nc.dram_tensor("wh0", [4, 128, 1024], bf16, kind="ExternalInput")
        wx1 = nc.dram_tensor("wx1", [4, 128, 1024], bf16, kind="ExternalInput")
        wh1 = nc.dram_tensor("wh1", [4, 128, 1024], bf16, kind="ExternalInput")
    wco = nc.dram_tensor("wco", [4, 128, 256], bf16, kind="ExternalInput")
    wcp = nc.dram_tensor("wcp", [4, 128, 256], bf16, kind="ExternalInput")
    fct = nc.dram_tensor("fct", [128, 128], bf16, kind="ExternalInput")
    cbias = nc.dram_tensor("cbias", [128, 2], f32, kind="ExternalInput")
    fbias = nc.dram_tensor("fbias", [64, 1], f32, kind="ExternalInput")
    if with_b1:
        b1row = nc.dram_tensor("b1row", [1, 1024], bf16, kind="ExternalInput")
    logT = nc.dram_tensor("logT", [64, TS * BC], bf16, kind="ExternalOutput")

    # internal dram
    logP = nc.dram_tensor("logP", [64, TS * BC], bf16)
    logS = nc.dram_tensor("logS", [64, TS * BC], bf16)
    hin = nc.dram_tensor("hin", [Tn + 1, 128, 512], bf16)
    hout = nc.dram_tensor("hout", [Tn + 1, 2, 128, 512], bf16)
    arout = nc.dram_tensor("arout", [TS, 2, 128, 512], bf16)

    if os.environ.get("BLSTM_NULL", "0") == "1":
        with tile.TileContext(nc) as tc, ExitStack() as ctx:
            pool = ctx.enter_context(tc.tile_pool(name="np", bufs=1))
            z = pool.tile([64, 512], bf16, name="z")
            nc.vector.memset(z, 0.0)
            nc.sync.dma_start(out=logT[:, 0:512], in_=z)
        nc.finalize()
        return nc

    with tile.TileContext(nc) as tc, ExitStack() as ctx:
        wpool = ctx.enter_context(tc.tile_pool(name="weights", bufs=1))
        spool = ctx.enter_context(tc.tile_pool(name="state", bufs=1))
        work = ctx.enter_context(tc.tile_pool(name="work", bufs=1))
        gpool = ctx.enter_context(tc.tile_pool(name="gp", bufs=1, space="PSUM"))
        auxp = ctx.enter_context(tc.tile_pool(name="auxp", bufs=1, space="PSUM"))

        # ---- load weights ----
        def load(dram, n, cols, tag):
            tiles = []
            for k in range(n):
                t_ = wpool.tile([128, cols], bf16, tag=f"{tag}{k}", name=f"{tag}{k}")
                nc.sync.dma_start(out=t_, in_=dram[k])
                tiles.append(t_)
            return tiles

        ohT_s = wpool.tile([64, (Tn + 1) * BC], bf16, tag="ohT")
        nc.sync.dma_start(out=ohT_s, in_=ohT[:, :])
        g0tab_s = wpool.tile([64, 1024], bf16, tag="g0tab")
        nc.sync.dma_start(out=g0tab_s, in_=g0tab[:, :])

        def load8(dram, n, cols, tag):
            tiles = []
            for k in range(n):
                t_ = wpool.tile([128, cols], fp8e4, tag=f"{tag}{k}", name=f"{tag}{k}")
                nc.sync.dma_start(out=t_, in_=dram[k])
                tiles.append(t_)
            return tiles

        if fp8:
            wh0_s = load8(wh0, 2, 2048, "wh0")
            wx1_s = load8(wx1, 2, 2048, "wx1")
            wh1_s = load8(wh1, 2, 2048, "wh1")
        else:
            wh0_s = load(wh0, 4, 1024, "wh0")
            wx1_s = load(wx1, 4, 1024, "wx1")
            wh1_s = load(wh1, 4, 1024, "wh1")
        wco_s = load(wco, 4, 256, "wco")
        wcp_s = load(wcp, 4, 256, "wcp")
        fct_s = wpool.tile([128, 128], bf16, tag="fct")
        nc.sync.dma_start(out=fct_s, in_=fct[:, :])
        cbias_s = wpool.tile([128, 2], f32, tag="cbias")
        nc.sync.dma_start(out=cbias_s, in_=cbias[:, :])
        fbias_s = wpool.tile([64, 1], f32, tag="fbias")
        nc.sync.dma_start(out=fbias_s, in_=fbias[:, :])
        if with_b1:
            b1_s = wpool.tile([1, 1024], bf16, tag="b1row")
            nc.sync.dma_start(out=b1_s, in_=b1row[:, :])
            ones_s = wpool.tile([1, 128], bf16, tag="ones")
            nc.vector.memset(ones_s, 1.0)

        # ---- state ----
        # X: [G0(256) c0(256) G1(256) c1(256)] fp32
        X = spool.tile([128, 1024], f32, tag="X")
        hT_ring = [spool.tile([128, 1024], bf16, tag=f"hT{i}", name=f"hT{i}")
                   for i in range(3)]
        if fp8:
            hT8_ring = [spool.tile([128, 1024], fp8e4, tag=f"hT8{i}",
                                   name=f"hT8{i}") for i in range(3)]

        def init_state():
            nc.vector.memset(X, 0.0)
            nc.vector.memset(hT_ring[2], 0.0)
            if fp8:
                nc.vector.memset(hT8_ring[2], 0.0)

        def gates_mms(gp, u, hT, mode="full"):
            """Gate matmuls: L0 gates (step u) into cols 0:1024,
            L1 gates (step u-1) into cols 1024:2048.
            mode="first": h(-1)=0 -> only x-part mms + memset L1 region.
            mode="last": L0 output unused -> memset L0 region, L1 mms only."""
            xstat = (ohT_s[:, u * BC:(u + 1) * BC], g0tab_s, None)
            if fp8:
                hv = hT.rearrange("p (j b) -> p j b", j=8)
                # DoubleRow pairs: h0 = blocks (0,1),(4,5); h1 = (2,3),(6,7)
                reg0 = [xstat]
                reg0 += [(hv[:, 4 * P:4 * P + 2, :], wh0_s[P], DR)
                         for P in (0, 1)]
                reg1 = [(hv[:, 4 * P:4 * P + 2, :], wx1_s[P], DR)
                        for P in (0, 1)]
                reg1 += [(hv[:, 4 * P + 2:4 * P + 4, :], wh1_s[P], DR)
                         for P in (0, 1)]
            else:
                reg0 = [xstat]
                reg0 += [(hT[:, BH0[kc] * 128:BH0[kc] * 128 + 128],
                          wh0_s[kc], None) for kc in range(4)]
                reg1 = [(hT[:, BH0[kc] * 128:BH0[kc] * 128 + 128],
                         wx1_s[kc], None) for kc in range(4)]
                reg1 += [(hT[:, BH1[kc] * 128:BH1[kc] * 128 + 128],
                          wh1_s[kc], None) for kc in range(4)]
            if with_b1:
                reg1.append((ones_s, b1_s, None))
            if mode == "first" and not with_b1:
                # x-part only; L1 gates (step -1) are all zero
                reg0 = [xstat]
                nc.vector.memset(gp[:, 1024:2048], 0.0)
                reg1 = []
            elif mode == "last":
                nc.vector.memset(gp[:, 0:1024], 0.0)
                reg0 = []
            for base, stats in ((0, reg0), (1024, reg1)):
                nk = len(stats)
                for kid, (lhs, w, pm) in enumerate(stats):
                    for n in (0, 1):
                        if pm is DR:
                            rhs = w.rearrange("p (t i n) -> p t i n",
                                              t=2, i=2)[:, n]
                        else:
                            rhs = w[:, 512 * n: 512 * n + 512]
                        nc.tensor.matmul(
                            gp[:, base + 512 * n: base + 512 * n + 512],
                            lhsT=lhs,
                            rhs=rhs,
                            start=(kid == 0),
                            stop=(kid == nk - 1),
                            perf_mode=pm,
                            tile_position=(0, 0),
                        )

        def cell(gp):
            """Fused 2-layer cell; returns H [128, 512] = [H0own|H1own]."""
            gv = gp.rearrange("p (j c) -> p j c", j=2)       # [128, 2, 1024]
            S = work.tile([128, 1536], bf16, tag="S")
            Sv = S.rearrange("p (j c) -> p j c", j=2)        # [128, 2, 768]
            nc.scalar.activation(Sv, gv[:, :, 0:768], AF.Sigmoid)
            Xv = X.rearrange("p (j c) -> p j c", j=2)        # [128, 2, 512]
            nc.scalar.activation(Xv[:, :, 0:256], gv[:, :, 768:1024], AF.Tanh)
            P = work.tile([128, 1024], f32, tag="P")
            Pv = P.rearrange("p (j c) -> p j c", j=2)        # [128, 2, 512]
            nc.vector.tensor_mul(Pv, Sv[:, :, 0:512], Xv)
            nc.vector.tensor_add(Xv[:, :, 256:512], Pv[:, :, 0:256],
                                 Pv[:, :, 256:512])
            TC = work.tile([128, 512], bf16, tag="TC")
            TCv = TC.rearrange("p (j c) -> p j c", j=2)      # [128, 2, 256]
            nc.scalar.activation(TCv, Xv[:, :, 256:512], AF.Tanh)
            H = work.tile([128, 512], bf16, tag="H")
            Hv = H.rearrange("p (j c) -> p j c", j=2)
            nc.vector.tensor_mul(Hv, Sv[:, :, 512:768], TCv)
            return H

        def emit_recurrence():
            init_state()
            for u in range(Tn + 1):
                gp = gpool.tile([128, 2048], f32, tag="gp", name="gp")
                ring = hT8_ring if fp8 else hT_ring
                mode = "first" if u == 0 else ("last" if u == Tn else "full")
                gates_mms(gp, u, ring[(u - 1) % 3] if u > 0 else ring[2], mode)
                H = cell(gp)
                nc.sync.dma_start(out=hin[u], in_=H)
                nc.gpsimd.collective_compute(
                    "AllGather", mybir.AluOpType.bypass,
                    replica_groups=GATE_PAIRS,
                    ins=[hin[u]], outs=[hout[u]])
                HF = work.tile([128, 1024], bf16, tag="HF")
                nc.sync.dma_start(out=HF, in_=hout[u].rearrange("j p c -> p j c"))
                nc.sync.dma_start_transpose(
                    out=hT_ring[u % 3].rearrange("p (j b) -> p j b", j=8),
                    in_=HF)
                if fp8:
                    nc.vector.tensor_copy(hT8_ring[u % 3], hT_ring[u % 3])

        def emit_tail():
            # AllReduce the late window: slots [TS+1, Tn+1) (TS slots).
            nc.gpsimd.collective_compute(
                "AllReduce", mybir.AluOpType.add,
                replica_groups=DIR_PAIRS,
                ins=[hin_window()], outs=[arout[:, :, :, :]])
            for i in range(NG):
                emit_tail_group(i)
            nc.gpsimd.collective_compute(
                "AllReduce", mybir.AluOpType.add,
                replica_groups=GATE_PAIRS,
                ins=[logP[:, :]], outs=[logS[:, :]])
            nc.sync.dma_start(out=logT[:, :], in_=logS[:, :])

        def hin_window():
            # copy hout slots [TS+1 : Tn+1) to arin via dram->dram dma?
            # cheaper: AllReduce directly on a hout slice.
            return hout[TS + 1: Tn + 1]

        def emit_tail_group(i):
            # own-scan steps s = 4i .. 4i+3; 512 rows.
            # Load FULL h slots [2,128,512] -> [128, 1024] per step
            # (h1 blocks selected post-transpose via BH1 strides).
            HT1 = work.tile([128, 4096], bf16, tag="HT1")   # own-dir rows
            ART = work.tile([128, 4096], bf16, tag="ART")   # fwd+bwd sum
            OWT = work.tile([128, 4096], bf16, tag="OWT")   # own window
            for k in range(4):
                s = 4 * i + k
                cs = slice(k * 1024, (k + 1) * 1024)
                rr = lambda t_: t_.rearrange("j p c -> p j c")
                nc.sync.dma_start(out=HT1[:, cs], in_=rr(hout[s + 1]))
                nc.sync.dma_start(out=ART[:, cs], in_=rr(arout[TS - 1 - s]))
                nc.sync.dma_start(out=OWT[:, cs], in_=rr(hout[Tn - s]))
            PRT = work.tile([128, 4096], bf16, tag="PRT")   # partner rows
            nc.vector.tensor_sub(PRT, ART, OWT)
            hTo = work.tile([128, 4096], bf16, tag="hTo")
            nc.sync.dma_start_transpose(
                out=hTo.rearrange("p (j b) -> p j b", j=32), in_=HT1)
            hTp = work.tile([128, 4096], bf16, tag="hTp")
            nc.sync.dma_start_transpose(
                out=hTp.rearrange("p (j b) -> p j b", j=32), in_=PRT)
            # compress matmuls for this core's comp-HALF: 2 cc chunks
            PT = auxp.tile([128, 1024], f32, tag="aux", name="PT")
            hToV = hTo.rearrange("p (s j b) -> p s j b", s=4, j=8)
            hTpV = hTp.rearrange("p (s j b) -> p s j b", s=4, j=8)
            for cc in range(2):
                for ki, (wt, hv) in enumerate(
                        [(wco_s, hToV), (wcp_s, hTpV)]):
                    for kc in range(4):
                        nc.tensor.matmul(
                            PT[:, cc * 512:(cc + 1) * 512],
                            lhsT=wt[kc][:, cc * 128:(cc + 1) * 128],
                            rhs=hv[:, :, BH1[kc], :],
                            start=(ki == 0 and kc == 0),
                            stop=(ki == 1 and kc == 3),
                            tile_position=(0, 0),
                        )
            C = work.tile([128, 1024], bf16, tag="C")
            for cc in range(2):
                nc.scalar.activation(
                    C[:, cc * 512:(cc + 1) * 512],
                    PT[:, cc * 512:(cc + 1) * 512],
                    AF.Tanh, bias=cbias_s[:, cc:cc + 1])
            lg = auxp.tile([64, 512], f32, tag="aux", name="lg")
            for cc in range(2):
                nc.tensor.matmul(
                    lg,
                    lhsT=fct_s[:, cc * 64:(cc + 1) * 64],
                    rhs=C[:, cc * 512:(cc + 1) * 512],
                    start=(cc == 0), stop=(cc == 1),
                    tile_position=(0, 0))
            lgs = work.tile([64, 512], bf16, tag="lgs")
            nc.scalar.activation(lgs, lg, AF.Identity, bias=fbias_s[:, 0:1])
            nc.sync.dma_start(out=logP[:, 512 * i: 512 * (i + 1)], in_=lgs)

        for _ in range(repeat):
            emit_recurrence()
            emit_tail()

    nc.finalize()
    return nc


_prog_cache = {}


def _get_program(key):
    if key not in _prog_cache:
        _prog_cache[key] = build_program(*key)
    return _prog_cache[key]


def _drpackc(w4):
    # [4, 128, 512] -> pairs [2, 128, 1024]: per cc: [A_cc(128) | B_cc(128)]
    FP8 = ml_dtypes.float8_e4m3
    out = np.empty((2, 128, 1024), np.float32)
    for P in (0, 1):
        A, Bc = w4[2 * P], w4[2 * P + 1]
        cols = []
        for cc in range(4):
            cols += [A[:, cc * 128:(cc + 1) * 128], Bc[:, cc * 128:(cc + 1) * 128]]
        out[P] = np.concatenate(cols, axis=1)
    return out.astype(FP8)


def _gate_perm_half(g):
    """Rows of W (gate dim 2048, blocks [i,f,g,o] of 512) for half g in
    cell order [i f o g] x hid [g*256,(g+1)*256)."""
    perm = []
    for blk in (0, 1, 3, 2):   # i, f, o, g
        base = 512 * blk + HH * g
        perm.extend(range(base, base + HH))
    return np.array(perm)


def _prep_core_inputs(x, emb_table, inputs, bh, d, g, t_steps=T, fp8=False, tail8=False):
    perm = _gate_perm_half(g)
    Tn = t_steps
    xq = np.asarray(x[bh * BC:(bh + 1) * BC, :Tn]).astype(np.int64)
    if d == 1:
        xq = xq[:, ::-1]
    xs = xq.T.reshape(-1)                       # [Tn*BC] scan-order tokens
    ohv = np.zeros((64, (Tn + 1) * BC), dtype=np.float32)
    ohv[xs, np.arange(Tn * BC)] = 1.0           # last step stays zero

    W0 = np.asarray(inputs["W_f0"] if d == 0 else inputs["W_b0"])
    b0 = np.asarray(inputs["b_f0"] if d == 0 else inputs["b_b0"])
    W1 = np.asarray(inputs["W_f1"] if d == 0 else inputs["W_b1"])
    b1 = np.asarray(inputs["b_f1"] if d == 0 else inputs["b_b1"])
    W0h = W0[perm].astype(np.float32)           # [1024, 768]
    W1h = W1[perm].astype(np.float32)           # [1024, 1024]
    g0v = np.asarray(emb_table, np.float32) @ W0h[:, :EMB].T + b0[perm][None, :]
    wh0v = W0h[:, EMB:].T.reshape(4, 128, 1024)
    wx1v = W1h[:, :HS].T.reshape(4, 128, 1024)
    wh1v = W1h[:, HS:].T.reshape(4, 128, 1024)

    Wc = np.asarray(inputs["compress_W"], np.float32)     # [512, 1024]
    wc_own = Wc[:, d * HS:(d + 1) * HS]                   # own direction
    wc_prt = Wc[:, (1 - d) * HS:(2 - d) * HS]
    # comp-HALF split across gate-half cores: this core handles comp
    # dims [g*256, (g+1)*256)
    wcov = wc_own.T.reshape(4, 128, 512)[:, :, g * 256:(g + 1) * 256]
    wcpv = wc_prt.T.reshape(4, 128, 512)[:, :, g * 256:(g + 1) * 256]
    wcov = np.ascontiguousarray(wcov)
    wcpv = np.ascontiguousarray(wcpv)

    if fp8:
        FP8 = ml_dtypes.float8_e4m3

        def drpack(w4):
            # [4, 128, 1024] -> pairs [2, 128, 2048]: [A n0 | B n0 | A n1 | B n1]
            out = np.empty((2, 128, 2048), np.float32)
            for P in (0, 1):
                A, Bc = w4[2 * P], w4[2 * P + 1]
                out[P] = np.concatenate(
                    [A[:, :512], Bc[:, :512], A[:, 512:], Bc[:, 512:]], axis=1)
            return out.astype(FP8)

        wh0v, wx1v, wh1v = drpack(wh0v), drpack(wx1v), drpack(wh1v)
    # fcT chunks for this core's comp-half: 2 chunks [128, 64] in cols
    fctv = np.ascontiguousarray(
        np.asarray(inputs["fc_W"], np.float32).T[g * 256:(g + 1) * 256]
        .reshape(2, 128, 64).transpose(1, 0, 2).reshape(128, 128))
    cbv = np.ascontiguousarray(
        np.asarray(inputs["compress_b"], np.float32)
        [g * 256:(g + 1) * 256].reshape(2, 128).T)
    # fc bias contributed once per gate-pair (g0 core only)
    fbv = (np.asarray(inputs["fc_b"], np.float32).reshape(64, 1)
           if g == 0 else np.zeros((64, 1), np.float32))

    wdt = (lambda a: a) if fp8 else (lambda a: a.astype(BF16))
    cdt = lambda a: a.astype(BF16)
    inmap = {
        "ohT": ohv.astype(BF16),
        "g0tab": g0v.astype(BF16),
        "wh0": wdt(wh0v),
        "wx1": wdt(wx1v),
        "wh1": wdt(wh1v),
        "wco": cdt(wcov),
        "wcp": cdt(wcpv),
        "fct": fctv.astype(BF16),
        "cbias": np.ascontiguousarray(cbv),
        "fbias": fbv,
    }
    if np.any(b1):
        inmap["b1row"] = b1[perm].reshape(1, 1024).astype(BF16)
    return inmap


def _run(inputs, trace=False, t_steps=T):
    from concourse.bass_utils import run_bass_kernel_spmd

    x = np.asarray(inputs["x"])
    emb_table = np.asarray(inputs["emb_table"], dtype=np.float32)
    with_b1 = bool(np.any(np.asarray(inputs["b_f1"]))
                   or np.any(np.asarray(inputs["b_b1"])))
    rep = int(os.environ.get("BLSTM_REPEAT", "1"))
    fp8 = os.environ.get("BLSTM_FP8", "0") == "1"
    tail8 = False  # fp8 tail removed (comp-split tail is bf16-only)
    nc = _get_program((t_steps, rep, with_b1, fp8, tail8))

    in_maps = []
    for core in range(NCORES):
        bh, d, g = core // 4, (core % 4) // 2, core % 2
        im = _prep_core_inputs(x, emb_table, inputs, bh, d, g, t_steps, fp8, tail8)
        if with_b1 and "b1row" not in im:
            im["b1row"] = np.zeros((1, 1024), dtype=BF16)
        in_maps.append(im)

    res = run_bass_kernel_spmd(nc, in_maps, core_ids=list(range(NCORES)),
                               trace=trace)

    Tn = t_steps
    TS = Tn // 2
    out = np.empty((B, Tn, VS), dtype=np.float32)
    for bh in (0, 1):
        logF = res.results[bh * 4 + 0]["logT"].astype(np.float32)
        logB = res.results[bh * 4 + 2]["logT"].astype(np.float32)
        bsl = slice(bh * BC, (bh + 1) * BC)
        # fwd core: scan u = orig t in [0, TS)
        out[bsl, :TS] = logF.reshape(VS, TS, BC).transpose(2, 1, 0)
        # bwd core: scan u = orig Tn-1-u; its rows cover orig [TS, Tn)
        ob = logB.reshape(VS, TS, BC).transpose(2, 1, 0)   # [BC, u, VS]
        out[bsl, TS:] = ob[:, ::-1]
    return out, res


def kernel(**inputs):
    out, _ = _run(inputs, trace=False)
    return out


def kernel_profiled(**inputs):
    out, res = _run(inputs, trace=True)
    return out, res
